# revision 12
# baseline (speedup 1.0000x reference)
"""CIoU kernel v4: op-cost-aware redesign from real-HW microbenchmarks.

Key changes vs v3:
  - guarded reciprocal via cody_waite(D, sgD, -eps) + reciprocal_approx_fast
    (replaces Sign/STT/full-reciprocal chain: 24us+7us -> 6.6us per chunk);
  - window trick reworked to clip[0,1] + +-2*sgn shift (B side runs in the
    -SA/D direction: clip[-1,0] + swapped min/max trees);
  - all strided TensorReduces (5.5us each) replaced by slice-halving
    tensor_tensor min/max/add trees (~1.5us each);
  - hull bridge masks via sign algebra: u = sAm - sAf + sBf - sBm in {-4..4},
    mB = u - clip(u, -3, 3) in {-1, 0, +1};
  - Act engine owns all Sign ops, Pool owns t2 + the three clip TSPs;
  - buffers aliased so the whole chunk fits ~198KB/partition SBUF.
"""
import sys

sys.path.insert(0, "/opt/trn_rl_repo")

import numpy as np
import concourse.bass as bass
import concourse.bacc as bacc
import concourse.tile as tile
from concourse import mybir
from concourse.bass_utils import run_bass_kernel_spmd

AOT = mybir.AluOpType
ACT = mybir.ActivationFunctionType
F32 = mybir.dt.float32
F16 = mybir.dt.float16

B = 262144
NCORES = 8
NI = B // NCORES
P = 128
EPS = 1e-6
TINY = 1e-30

ASSIGN = {}


def build_program(T=64, assign=None, npass=1):
    global ASSIGN
    ASSIGN = dict(assign or {})
    CH = P * T
    NCH = NI // CH
    nc = bacc.Bacc("TRN2", target_bir_lowering=False, debug=False, num_devices=NCORES)
    ab_d = nc.dram_tensor("ab", [NI, 32], F32, kind="ExternalInput")
    out_d = nc.dram_tensor("ciou", [NI], F32, kind="ExternalOutput")

    with tile.TileContext(nc) as tc:
        with tc.tile_pool(name="pool", bufs=1) as pool, \
             tc.tile_pool(name="spool", bufs=1) as spool:
            tb = spool.tile([P, 1], F32, tag="tinyb", name="tinyb")
            nc.gpsimd.memset(tb, TINY)
            tm2 = spool.tile([P, 1], F32, tag="tm2b", name="tm2b")
            nc.gpsimd.memset(tm2, -2.0)
            for p_i in range(npass):
                for ch in range(NCH):
                    _chunk(nc, pool, spool, ab_d, out_d, ch, T, tb, tm2)
    nc.compile()
    return nc


def _chunk(nc, pool, spool, ab_d, out_d, ch, T, tb, tm2):
    CH = P * T
    v = nc.vector
    g = nc.gpsimd
    s = nc.scalar

    def E(site, default):
        return {"v": v, "g": g, "s": s}[ASSIGN.get(site, default)]

    def big(tag, d=F32, n=64):
        return pool.tile([P, n * T], d, tag=tag, name=tag)

    def small(tag, d=F32, n=8):
        return spool.tile([P, n * T], d, tag=tag, name=tag)

    def tiny(tag, d=F32):
        return spool.tile([P, T], d, tag=tag, name=tag)

    def dn(tl):
        return tl.rearrange("p (i k t) -> p i k t", i=8, k=8)

    # ---------------- load ----------------
    raw = pool.tile([P, 32 * T], F32, tag="raw", name="raw")
    ab_view = ab_d[ch * CH:(ch + 1) * CH, :].rearrange("(p t) jc -> p (t jc)", p=P)
    nc.sync.dma_start(raw, ab_view)

    rr = raw.rearrange("p (t h j c) -> p h j c t", h=2, j=8, c=2)
    ax = rr[:, 0, :, 0, :]; ay = rr[:, 0, :, 1, :]
    bx = rr[:, 1, :, 0, :]; by = rr[:, 1, :, 1, :]

    # ---------------- adjacency (both polygons as (g=2, s=8) groups) -------
    rr16 = raw.rearrange("p (t h j c) -> p h j c t", h=2, j=8, c=2)
    xs = rr16.rearrange("p h j c t -> p (h j) c t")[:, :, 0, :]
    ys = rr16.rearrange("p h j c t -> p (h j) c t")[:, :, 1, :]
    xg = xs.rearrange("p (g s) t -> p g s t", g=2)
    yg = ys.rearrange("p (g s) t -> p g s t", g=2)
    va1 = small("va1", n=16); va2 = small("va2", n=16)
    m1 = va1.rearrange("p (g s t) -> p g s t", g=2, s=8)
    m2 = va2.rearrange("p (g s t) -> p g s t", g=2, s=8)
    v.tensor_tensor(m1[:, :, 0:7, :], xg[:, :, 0:7, :], yg[:, :, 1:8, :], AOT.mult)
    v.tensor_tensor(m1[:, :, 7, :], xg[:, :, 7, :], yg[:, :, 0, :], AOT.mult)
    v.tensor_tensor(m2[:, :, 0:7, :], yg[:, :, 0:7, :], xg[:, :, 1:8, :], AOT.mult)
    v.tensor_tensor(m2[:, :, 7, :], yg[:, :, 7, :], xg[:, :, 0, :], AOT.mult)
    adjAB = small("adjab", n=16)
    adjAB_v = adjAB.rearrange("p (s t) -> p s t", s=16)
    adjA_c = adjAB_v[:, 0:8, :]
    adjB_c = adjAB_v[:, 8:16, :]
    v.tensor_tensor(adjAB, va1, va2, AOT.subtract)

    adjA_ik = adjA_c.unsqueeze(2).broadcast_to((P, 8, 8, T))
    adjB_ik = adjB_c.unsqueeze(1).broadcast_to((P, 8, 8, T))

    # ---------------- C (9x9 padded) ----------------
    A = big("bufA")          # rotating 64T scratch
    Bb = big("bufB")
    ax_b = ax.unsqueeze(2).broadcast_to((P, 8, 8, T))
    ay_b = ay.unsqueeze(2).broadcast_to((P, 8, 8, T))
    bx_b = bx.unsqueeze(1).broadcast_to((P, 8, 8, T))
    by_b = by.unsqueeze(1).broadcast_to((P, 8, 8, T))
    E("t1", "v").tensor_tensor(dn(A), ax_b, by_b, AOT.mult)
    E("t2", "v").tensor_tensor(dn(Bb), ay_b, bx_b, AOT.mult)

    cpad = big("cpad", n=81)
    cp = cpad.rearrange("p (si sk t) -> p si sk t", si=9, sk=9)
    E("c", "v").tensor_tensor(cp[:, 0:8, 0:8, :], dn(A), dn(Bb), AOT.subtract)
    if ASSIGN.get("pads", "v") == "s":
        s.copy(cp[:, 0:8, 8, :], cp[:, 0:8, 0, :])
        s.copy(cp[:, 8, :, :], cp[:, 0, :, :])
    else:
        v.tensor_scalar(cp[:, 0:8, 8, :], cp[:, 0:8, 0, :], 1.0, None, AOT.mult)
        v.tensor_scalar(cp[:, 8, :, :], cp[:, 0, :, :], 1.0, None, AOT.mult)

    # ---------------- w1 / D / SA / w2 / SB ----------------
    w1p = big("w1p", n=72)
    w1v = w1p.rearrange("p (i q t) -> p i q t", i=8, q=9)
    E("w1", "v").tensor_tensor(w1v, cp[:, 1:9, :, :], cp[:, 0:8, :, :], AOT.subtract)

    D = big("bufD")
    E("d", "v").tensor_tensor(dn(D), w1v[:, :, 1:9, :], w1v[:, :, 0:8, :], AOT.subtract)

    gD4 = big("sgd")
    E("gd", "v").tensor_scalar(gD4, D, 0.0, 4.0, AOT.is_ge, AOT.mult)  # 4*[D>=0]

    sap = big("sap", n=72)                          # SA 9 i-slots (SA at 1:9)
    sav = sap.rearrange("p (si k t) -> p si k t", si=9, k=8)
    SA = sav[:, 1:9, :, :]
    E("sa", "v").tensor_tensor(SA, w1v[:, :, 0:8, :], adjA_ik, AOT.add)
    if ASSIGN.get("pads", "v") == "s":
        s.copy(sav[:, 0, :, :], sav[:, 8, :, :])
    else:
        v.tensor_scalar(sav[:, 0, :, :], sav[:, 8, :, :], 1.0, None, AOT.mult)

    W2 = A                                          # reuse bufA
    E("w2", "v").tensor_tensor(dn(W2), cp[:, 0:8, 0:8, :], cp[:, 0:8, 1:9, :], AOT.subtract)
    sbp = big("sbp", n=72)                          # SB 9 k-slots (SB at 1:9)
    sbv = sbp.rearrange("p (i sk t) -> p i sk t", i=8, sk=9)
    SB = sbv[:, :, 1:9, :]
    E("sb", "v").tensor_tensor(SB, dn(W2), adjB_ik, AOT.add)
    if ASSIGN.get("pads", "v") == "s":
        s.copy(sbv[:, :, 0, :], sbv[:, :, 8, :])
    else:
        v.tensor_scalar(sbv[:, :, 0, :], sbv[:, :, 8, :], 1.0, None, AOT.mult)

    # ---------------- f16 {0,1} masks of SA / SB ----------------
    saf = pool.tile([P, 72 * T], F16, tag="saf", name="saf")
    safv = saf.rearrange("p (si k t) -> p si k t", si=9, k=8)
    E("pa", "v").tensor_scalar(saf, sap, 0.0, None, AOT.is_ge)
    sbf = pool.tile([P, 72 * T], F16, tag="sbf", name="sbf")
    sbfv = sbf.rearrange("p (i sk t) -> p i sk t", i=8, sk=9)
    E("pb", "v").tensor_scalar(sbf, sbp, 0.0, None, AOT.is_ge)

    # ---------------- guarded reciprocal ----------------
    Dsafe = Bb                                      # reuse bufB
    v.affine_then_add(Dsafe, gD4, D, scale=EPS / 2, bias=-EPS)  # D + EPS*sgn
    R = A                                           # bufA dead after SB
    v.reciprocal_approx_fast(out=R, in_=Dsafe)

    # ---------------- A-side windows: ratio=SB/D, rc=clip01, q=rc-2sg ------
    ratioA = Bb                                     # Dsafe dead after R
    E("ra", "v").tensor_tensor(dn(ratioA), SB, dn(R), AOT.mult)
    rcA = D                                         # D dead after Dsafe/sgD
    E("rca", "v").tensor_scalar(rcA, ratioA, 0.0, 1.0, AOT.max, AOT.min)
    qA = w1p[:, 0:64 * T]                           # w1 dead after D/SA
    v.cody_waite_cascade(qA, rcA, gD4, 1.0, 0.0, 0.0)    # rc - gD4
    qAv = qA.rearrange("p (i k t) -> p i k t", i=8, k=8)

    # trees over k into unified 16-slot lo/hi (A: slots 0:8, B: 8:16)
    # scratch lives in buffers dead at tree time (NOT raw, so the next
    # chunk's input DMA can issue as soon as construction reads finish)
    t32a = D[:, 0:32 * T].rearrange("p (i k t) -> p i k t", i=8, k=4)
    mid = small("mid", n=16)
    m16 = mid.rearrange("p (i k t) -> p i k t", i=8, k=2)
    lo16 = small("lo16", n=16)
    hi16 = small("hi16", n=16)
    lo16v = lo16.rearrange("p (s t) -> p s t", s=16)
    hi16v = hi16.rearrange("p (s t) -> p s t", s=16)
    v.tensor_tensor(t32a, qAv[:, :, 0:4, :], qAv[:, :, 4:8, :], AOT.max)
    v.tensor_tensor(m16, t32a[:, :, 0:2, :], t32a[:, :, 2:4, :], AOT.max)
    v.tensor_tensor(lo16v[:, 0:8, :], m16[:, :, 0, :], m16[:, :, 1, :], AOT.max)
    v.tensor_tensor(t32a, qAv[:, :, 0:4, :], qAv[:, :, 4:8, :], AOT.min)
    v.tensor_tensor(m16, t32a[:, :, 0:2, :], t32a[:, :, 2:4, :], AOT.min)
    v.tensor_tensor(hi16v[:, 0:8, :], m16[:, :, 0, :], m16[:, :, 1, :], AOT.min)

    # ---------------- B-side windows (negated at write: q = -(rc'-2sg)) ----
    ratioB = sbp[:, 0:64 * T]                       # SB dead after ratioA & sbf
    E("rb", "v").tensor_tensor(
        ratioB.rearrange("p (i k t) -> p i k t", i=8, k=8), SA, dn(R), AOT.mult)
    rcBp = D                                        # rcA dead after qA
    E("rcb", "v").tensor_scalar(rcBp, ratioB, -1.0, 0.0, AOT.max, AOT.min)
    qB = Bb                                         # ratioA dead after rcA
    v.ln_bwd_dx(qB, rcBp, gD4, 1.0, -4.0, -1.0)     # qB = -rc' + gD4 - 4
    qBv = qB.rearrange("p (i k t) -> p i k t", i=8, k=8)

    # trees over i (same polarity as A side now)
    t32b = gD4[:, 0:32 * T].rearrange("p (i k t) -> p i k t", i=4, k=8)
    m16b = mid.rearrange("p (i k t) -> p i k t", i=2, k=8)
    v.tensor_tensor(t32b, qBv[:, 0:4, :, :], qBv[:, 4:8, :, :], AOT.max)
    v.tensor_tensor(m16b, t32b[:, 0:2, :, :], t32b[:, 2:4, :, :], AOT.max)
    v.tensor_tensor(lo16v[:, 8:16, :], m16b[:, 0, :, :], m16b[:, 1, :, :], AOT.max)
    v.tensor_tensor(t32b, qBv[:, 0:4, :, :], qBv[:, 4:8, :, :], AOT.min)
    v.tensor_tensor(m16b, t32b[:, 0:2, :, :], t32b[:, 2:4, :, :], AOT.min)
    v.tensor_tensor(hi16v[:, 8:16, :], m16b[:, 0, :, :], m16b[:, 1, :, :], AOT.min)

    # unified widths: w = max(min(hi+2,1) - max(lo-2,0), 0); iw = w*adj
    x1 = small("x1", n=16)
    x2 = small("x2", n=16)
    iw = small("iw16", n=16)
    x3 = iw                    # WAR-safe: iw's own write reads only x4/adjAB
    x4 = small("x4", n=16)
    E("w1p", "v").tensor_scalar(x1, lo16, 0.0, None, AOT.max)   # lo2
    v.tensor_scalar(x2, hi16, 4.0, 1.0, AOT.add, AOT.min)       # hi2
    v.tensor_tensor(x3, x2, x1, AOT.subtract)
    v.tensor_scalar(x4, x3, 0.0, None, AOT.max)
    iwv = iw.rearrange("p (s t) -> p s t", s=16)
    v.tensor_tensor(iw, x4, adjAB, AOT.mult)
    isum = tiny("isum")
    v.tensor_reduce(isum, iw.rearrange("p (s t) -> p t s", s=16),
                    axis=mybir.AxisListType.X, op=AOT.add)

    # ---------------- hull: bridges via sign algebra ----------------
    sAm = safv[:, 0:8, :, :]; sAf = safv[:, 1:9, :, :]
    sBm = sbfv[:, :, 0:8, :]; sBf = sbfv[:, :, 1:9, :]
    s1 = R                                          # R dead after ratioB
    E("s1", "v").tensor_tensor(dn(s1), sAm, sBf, AOT.add)
    s2 = sap[:, 0:64 * T]                           # SA dead after ratioB & saf
    E("s2", "v").tensor_tensor(s2.rearrange("p (i k t) -> p i k t", i=8, k=8),
                               sAf, sBm, AOT.add)
    u = gD4                                         # gD4 dead after qB
    E("u", "v").tensor_tensor(u, s1, s2, AOT.subtract)
    c2t = D                                         # rcBp dead after qB
    E("c2", "v").tensor_scalar(c2t, u, 1.0, -1.0, AOT.min, AOT.max)
    dd = Bb                                         # qB dead after trees
    E("dd", "v").tensor_tensor(dd, u, c2t, AOT.subtract)
    cM = s1                                         # s1 dead after u
    E("cm", "v").tensor_tensor(dn(cM), dd.rearrange("p (i k t) -> p i k t", i=8, k=8),
                               cp[:, 0:8, 0:8, :], AOT.mult)
    # redBR = sum_{i,k} cM  (tree + small reduce)
    cMv = cM.rearrange("p (i k t) -> p i k t", i=8, k=8)
    t32r = D[:, 0:32 * T].rearrange("p (i k t) -> p i k t", i=8, k=4)
    v.tensor_tensor(t32r, cMv[:, :, 0:4, :], cMv[:, :, 4:8, :], AOT.add)
    v.tensor_tensor(m16, t32r[:, :, 0:2, :], t32r[:, :, 2:4, :], AOT.add)
    r8 = lo16[:, 0:8 * T]                           # reuse small
    v.tensor_tensor(r8.rearrange("p (i t) -> p i t", i=8),
                    m16[:, :, 0, :], m16[:, :, 1, :], AOT.add)
    redBR = tiny("redbr")
    v.tensor_reduce(redBR, r8.rearrange("p (s t) -> p t s", s=8),
                    axis=mybir.AxisListType.X, op=AOT.add)

    # ---------------- hull: surviving edges ----------------
    # okEA = all_k sAf >= 0  ->  min-tree over k of sAf (f16 +-1)
    tr16a = gD4[:, 32 * T:48 * T].bitcast(F16).rearrange("p (i k t) -> p i k t", i=8, k=4)
    md16 = mid[:, 0:8 * T].bitcast(F16).rearrange("p (i k t) -> p i k t", i=8, k=2)
    mnsA = va1.bitcast(F16)[:, 0:8 * T]             # va1 dead after adjacency
    v.tensor_tensor(tr16a, sAf[:, :, 0:4, :], sAf[:, :, 4:8, :], AOT.min)
    v.tensor_tensor(md16, tr16a[:, :, 0:2, :], tr16a[:, :, 2:4, :], AOT.min)
    v.tensor_tensor(mnsA.rearrange("p (i t) -> p i t", i=8),
                    md16[:, :, 0, :], md16[:, :, 1, :], AOT.min)
    hw = iw                                         # reuse iw16 after isum
    hwv = hw.rearrange("p (s t) -> p s t", s=16)
    v.tensor_tensor(hwv[:, 0:8, :], mnsA.rearrange("p (i t) -> p i t", i=8),
                    adjA_c, AOT.mult)
    tr16b = gD4[:, 32 * T:48 * T].bitcast(F16).rearrange("p (i k t) -> p i k t", i=4, k=8)
    md16b = mid[:, 0:8 * T].bitcast(F16).rearrange("p (i k t) -> p i k t", i=2, k=8)
    v.tensor_tensor(tr16b, sBf[:, 0:4, :, :], sBf[:, 4:8, :, :], AOT.min)
    v.tensor_tensor(md16b, tr16b[:, 0:2, :, :], tr16b[:, 2:4, :, :], AOT.min)
    v.tensor_tensor(mnsA.rearrange("p (k t) -> p k t", k=8),
                    md16b[:, 0, :, :], md16b[:, 1, :, :], AOT.min)
    v.tensor_tensor(hwv[:, 8:16, :], mnsA.rearrange("p (k t) -> p k t", k=8),
                    adjB_c, AOT.mult)
    h1 = tiny("h1")
    v.tensor_reduce(h1, hw.rearrange("p (s t) -> p t s", s=16),
                    axis=mybir.AxisListType.X, op=AOT.add)

    # ---------------- per-item finals ----------------
    asum = tiny("asum")
    v.tensor_reduce(asum, adjAB.rearrange("p (s t) -> p t s", s=16),
                    axis=mybir.AxisListType.X, op=AOT.add)
    hsum = tiny("hsum")
    v.tensor_tensor(hsum, h1, redBR, AOT.add)       # = 2*hull_area
    u2 = tiny("u2")
    v.tensor_tensor(u2, asum, isum, AOT.subtract)   # = 2*union
    ru2 = tiny("ru2")
    v.reciprocal_approx_fast(out=ru2, in_=u2)
    rh = tiny("rh")
    v.reciprocal_approx_fast(out=rh, in_=hsum)
    iou = tiny("iou")
    v.tensor_tensor(iou, isum, ru2, AOT.mult)
    t3 = tiny("t3")
    v.tensor_tensor(t3, u2, rh, AOT.mult)
    ciou = tiny("ciou")
    v.affine_then_add(ciou, iou, t3, scale=1.0, bias=-1.0)
    out_view = out_d[ch * CH:(ch + 1) * CH].rearrange("(p t) -> p t", p=P)
    nc.sync.dma_start(out_view, ciou)


_CACHE = {}
_ASSIGN_DEFAULT = {"pads": "s"}


def _get_executable(npass=1):
    key = ("exec", npass)
    if key in _CACHE:
        return _CACHE[key]
    import jax
    from jax.sharding import Mesh, PartitionSpec, NamedSharding
    from jax.experimental.shard_map import shard_map
    from concourse import bass2jax

    nc = build_program(assign=_ASSIGN_DEFAULT, npass=npass)
    bass2jax.install_neuronx_cc_hook()

    partition_name = nc.partition_id_tensor.name if nc.partition_id_tensor else None
    in_names, in_shapes, out_names, out_avals = [], [], [], []
    for alloc in nc.m.functions[0].allocations:
        if not isinstance(alloc, mybir.MemoryLocationSet):
            continue
        name = alloc.memorylocations[0].name
        if alloc.kind == "ExternalInput":
            if name != partition_name:
                in_names.append(name)
                in_shapes.append((tuple(alloc.tensor_shape), mybir.dt.np(alloc.dtype)))
        elif alloc.kind == "ExternalOutput":
            out_names.append(name)
            out_avals.append(jax.core.ShapedArray(
                tuple(alloc.tensor_shape), mybir.dt.np(alloc.dtype)))
    all_names = in_names + out_names
    if partition_name is not None:
        all_names = all_names + [partition_name]

    def _body(*args):
        operands = list(args)
        if partition_name is not None:
            operands.append(bass2jax.partition_id_tensor())
        outs = bass2jax._bass_exec_p.bind(
            *operands,
            out_avals=tuple(out_avals),
            in_names=tuple(all_names),
            out_names=tuple(out_names),
            lowering_input_output_aliases=(),
            sim_require_finite=True,
            sim_require_nnan=True,
            nc=nc,
        )
        return tuple(outs)

    devices = jax.devices()[:NCORES]
    mesh = Mesh(np.asarray(devices), ("core",))
    nin = len(in_names)
    nout = len(out_names)
    sh = NamedSharding(mesh, PartitionSpec("core"))
    jf = shard_map(_body, mesh=mesh,
                   in_specs=(PartitionSpec("core"),) * (nin + nout),
                   out_specs=(PartitionSpec("core"),) * nout,
                   check_rep=False)

    def compile_fn():
        args = [jax.ShapeDtypeStruct((NCORES * s[0], *s[1:]), d, sharding=sh)
                for (s, d) in in_shapes]
        args += [jax.ShapeDtypeStruct((NCORES * av.shape[0], *av.shape[1:]),
                                      av.dtype, sharding=sh)
                 for av in out_avals]
        return jax.jit(jf, keep_unused=True).lower(*args).compile()

    try:
        sharded = bass2jax.fast_dispatch_compile(compile_fn)
    except Exception:
        sharded = jax.jit(jf, keep_unused=True)
    zeros = [np.zeros((NCORES * av.shape[0], *av.shape[1:]), av.dtype)
             for av in out_avals]
    _CACHE[key] = (sharded, sh, zeros)
    return _CACHE[key]


def kernel(a: np.ndarray, b: np.ndarray) -> np.ndarray:
    import jax
    a8 = np.asarray(a, dtype=np.float32).reshape(NCORES, NI, 16)
    b8 = np.asarray(b, dtype=np.float32).reshape(NCORES, NI, 16)
    ab = np.ascontiguousarray(np.concatenate([a8, b8], axis=2))
    sharded, sh, zeros = _get_executable()
    ab_dev = jax.device_put(ab.reshape(NCORES * NI, 32), sh)
    zeros_dev = [jax.device_put(z, sh) for z in zeros]
    import time as _time
    ciou = None
    for attempt, delay in enumerate((0, 5, 20)):
        if delay:
            _time.sleep(delay)
        try:
            out = sharded(ab_dev, *zeros_dev)
            ciou = np.asarray(out[0], dtype=np.float64)
            break
        except Exception:
            if attempt == 2:
                raise
    return np.float32(ciou.sum() / B)


# revision 14
# speedup vs baseline: 1.0082x; 1.0082x over previous
"""CIoU kernel v4: op-cost-aware redesign from real-HW microbenchmarks.

Key changes vs v3:
  - guarded reciprocal via cody_waite(D, sgD, -eps) + reciprocal_approx_fast
    (replaces Sign/STT/full-reciprocal chain: 24us+7us -> 6.6us per chunk);
  - window trick reworked to clip[0,1] + +-2*sgn shift (B side runs in the
    -SA/D direction: clip[-1,0] + swapped min/max trees);
  - all strided TensorReduces (5.5us each) replaced by slice-halving
    tensor_tensor min/max/add trees (~1.5us each);
  - hull bridge masks via sign algebra: u = sAm - sAf + sBf - sBm in {-4..4},
    mB = u - clip(u, -3, 3) in {-1, 0, +1};
  - Act engine owns all Sign ops, Pool owns t2 + the three clip TSPs;
  - buffers aliased so the whole chunk fits ~198KB/partition SBUF.
"""
import sys

sys.path.insert(0, "/opt/trn_rl_repo")

import numpy as np
import concourse.bass as bass
import concourse.bacc as bacc
import concourse.tile as tile
from concourse import mybir
from concourse.bass_utils import run_bass_kernel_spmd

AOT = mybir.AluOpType
ACT = mybir.ActivationFunctionType
F32 = mybir.dt.float32
F16 = mybir.dt.float16

B = 262144
NCORES = 8
NI = B // NCORES
P = 128
EPS = 1e-6
TINY = 1e-30

ASSIGN = {}


def build_program(T=64, assign=None, npass=1):
    global ASSIGN
    ASSIGN = dict(assign or {})
    CH = P * T
    NCH = NI // CH
    nc = bacc.Bacc("TRN2", target_bir_lowering=False, debug=False, num_devices=NCORES)
    ab_d = nc.dram_tensor("ab", [NI, 32], F32, kind="ExternalInput")
    out_d = nc.dram_tensor("ciou", [NI], F32, kind="ExternalOutput")

    with tile.TileContext(nc) as tc:
        with tc.tile_pool(name="pool", bufs=1) as pool, \
             tc.tile_pool(name="spool", bufs=1) as spool:
            tb = spool.tile([P, 1], F32, tag="tinyb", name="tinyb")
            nc.gpsimd.memset(tb, TINY)
            tm2 = spool.tile([P, 1], F32, tag="tm2b", name="tm2b")
            nc.gpsimd.memset(tm2, -2.0)
            for p_i in range(npass):
                for ch in range(NCH):
                    _chunk(nc, pool, spool, ab_d, out_d, ch, T, tb, tm2)
    nc.compile()
    return nc


def _chunk(nc, pool, spool, ab_d, out_d, ch, T, tb, tm2):
    CH = P * T
    v = nc.vector
    g = nc.gpsimd
    s = nc.scalar

    def E(site, default):
        return {"v": v, "g": g, "s": s}[ASSIGN.get(site, default)]

    def big(tag, d=F32, n=64):
        return pool.tile([P, n * T], d, tag=tag, name=tag)

    def small(tag, d=F32, n=8):
        return spool.tile([P, n * T], d, tag=tag, name=tag)

    def tiny(tag, d=F32):
        return spool.tile([P, T], d, tag=tag, name=tag)

    def dn(tl):
        return tl.rearrange("p (i k t) -> p i k t", i=8, k=8)

    # ---------------- load ----------------
    raw = pool.tile([P, 32 * T], F32, tag="raw", name="raw")
    ab_view = ab_d[ch * CH:(ch + 1) * CH, :].rearrange("(p t) jc -> p (t jc)", p=P)
    nc.sync.dma_start(raw, ab_view)

    rr = raw.rearrange("p (t h j c) -> p h j c t", h=2, j=8, c=2)
    ax = rr[:, 0, :, 0, :]; ay = rr[:, 0, :, 1, :]
    bx = rr[:, 1, :, 0, :]; by = rr[:, 1, :, 1, :]

    # ---------------- adjacency (both polygons as (g=2, s=8) groups) -------
    rr16 = raw.rearrange("p (t h j c) -> p h j c t", h=2, j=8, c=2)
    xs = rr16.rearrange("p h j c t -> p (h j) c t")[:, :, 0, :]
    ys = rr16.rearrange("p h j c t -> p (h j) c t")[:, :, 1, :]
    xg = xs.rearrange("p (g s) t -> p g s t", g=2)
    yg = ys.rearrange("p (g s) t -> p g s t", g=2)
    va1 = small("va1", n=16); va2 = small("va2", n=16)
    m1 = va1.rearrange("p (g s t) -> p g s t", g=2, s=8)
    m2 = va2.rearrange("p (g s t) -> p g s t", g=2, s=8)
    v.tensor_tensor(m1[:, :, 0:7, :], xg[:, :, 0:7, :], yg[:, :, 1:8, :], AOT.mult)
    v.tensor_tensor(m1[:, :, 7, :], xg[:, :, 7, :], yg[:, :, 0, :], AOT.mult)
    v.tensor_tensor(m2[:, :, 0:7, :], yg[:, :, 0:7, :], xg[:, :, 1:8, :], AOT.mult)
    v.tensor_tensor(m2[:, :, 7, :], yg[:, :, 7, :], xg[:, :, 0, :], AOT.mult)
    adjAB = small("adjab", n=16)
    adjAB_v = adjAB.rearrange("p (s t) -> p s t", s=16)
    adjA_c = adjAB_v[:, 0:8, :]
    adjB_c = adjAB_v[:, 8:16, :]
    v.tensor_tensor(adjAB, va1, va2, AOT.subtract)

    adjA_ik = adjA_c.unsqueeze(2).broadcast_to((P, 8, 8, T))
    adjB_ik = adjB_c.unsqueeze(1).broadcast_to((P, 8, 8, T))

    # ---------------- C (9x9 padded) ----------------
    A = big("bufA")          # rotating 64T scratch
    Bb = big("bufB")
    ax_b = ax.unsqueeze(2).broadcast_to((P, 8, 8, T))
    ay_b = ay.unsqueeze(2).broadcast_to((P, 8, 8, T))
    bx_b = bx.unsqueeze(1).broadcast_to((P, 8, 8, T))
    by_b = by.unsqueeze(1).broadcast_to((P, 8, 8, T))
    E("t1", "v").tensor_tensor(dn(A), ax_b, by_b, AOT.mult)
    E("t2", "v").tensor_tensor(dn(Bb), ay_b, bx_b, AOT.mult)

    cpad = big("cpad", n=81)
    cp = cpad.rearrange("p (si sk t) -> p si sk t", si=9, sk=9)
    E("c", "v").tensor_tensor(cp[:, 0:8, 0:8, :], dn(A), dn(Bb), AOT.subtract)
    if ASSIGN.get("pads", "v") == "s":
        s.copy(cp[:, 0:8, 8, :], cp[:, 0:8, 0, :])
        s.copy(cp[:, 8, :, :], cp[:, 0, :, :])
    else:
        v.tensor_scalar(cp[:, 0:8, 8, :], cp[:, 0:8, 0, :], 1.0, None, AOT.mult)
        v.tensor_scalar(cp[:, 8, :, :], cp[:, 0, :, :], 1.0, None, AOT.mult)

    # ---------------- w1 / D / SA / w2 / SB ----------------
    w1p = big("w1p", n=72)
    w1v = w1p.rearrange("p (i q t) -> p i q t", i=8, q=9)
    E("w1", "v").tensor_tensor(w1v, cp[:, 1:9, :, :], cp[:, 0:8, :, :], AOT.subtract)

    D = big("bufD")
    E("d", "v").tensor_tensor(dn(D), w1v[:, :, 1:9, :], w1v[:, :, 0:8, :], AOT.subtract)

    gD4 = big("sgd")
    E("gd", "v").tensor_scalar(gD4, D, 0.0, 4.0, AOT.is_ge, AOT.mult)  # 4*[D>=0]

    sap = big("sap", n=72)                          # SA 9 i-slots (SA at 1:9)
    sav = sap.rearrange("p (si k t) -> p si k t", si=9, k=8)
    SA = sav[:, 1:9, :, :]
    E("sa", "v").tensor_tensor(SA, w1v[:, :, 0:8, :], adjA_ik, AOT.add)
    if ASSIGN.get("pads", "v") == "s":
        s.copy(sav[:, 0, :, :], sav[:, 8, :, :])
    else:
        v.tensor_scalar(sav[:, 0, :, :], sav[:, 8, :, :], 1.0, None, AOT.mult)

    W2 = A                                          # reuse bufA
    E("w2", "v").tensor_tensor(dn(W2), cp[:, 0:8, 0:8, :], cp[:, 0:8, 1:9, :], AOT.subtract)
    sbp = big("sbp", n=72)                          # SB 9 k-slots (SB at 1:9)
    sbv = sbp.rearrange("p (i sk t) -> p i sk t", i=8, sk=9)
    SB = sbv[:, :, 1:9, :]
    E("sb", "v").tensor_tensor(SB, dn(W2), adjB_ik, AOT.add)
    if ASSIGN.get("pads", "v") == "s":
        s.copy(sbv[:, :, 0, :], sbv[:, :, 8, :])
    else:
        v.tensor_scalar(sbv[:, :, 0, :], sbv[:, :, 8, :], 1.0, None, AOT.mult)

    # ---------------- f16 {0,1} masks of SA / SB ----------------
    saf = pool.tile([P, 72 * T], F16, tag="saf", name="saf")
    safv = saf.rearrange("p (si k t) -> p si k t", si=9, k=8)
    E("pa", "v").tensor_scalar(saf, sap, 0.0, None, AOT.is_ge)
    sbf = pool.tile([P, 72 * T], F16, tag="sbf", name="sbf")
    sbfv = sbf.rearrange("p (i sk t) -> p i sk t", i=8, sk=9)
    E("pb", "v").tensor_scalar(sbf, sbp, 0.0, None, AOT.is_ge)

    # ---------------- guarded reciprocal ----------------
    Dsafe = Bb                                      # reuse bufB
    v.affine_then_add(Dsafe, gD4, D, scale=EPS / 2, bias=-EPS)  # D + EPS*sgn
    R = A                                           # bufA dead after SB
    v.reciprocal_approx_fast(out=R, in_=Dsafe)

    # ---------------- A-side windows: ratio=SB/D, rc=clip01, q=rc-2sg ------
    ratioA = Bb                                     # Dsafe dead after R
    E("ra", "v").tensor_tensor(dn(ratioA), SB, dn(R), AOT.mult)
    rcA = D                                         # D dead after Dsafe/sgD
    E("rca", "v").tensor_scalar(rcA, ratioA, 0.0, 1.0, AOT.max, AOT.min)
    qA = w1p[:, 0:64 * T]                           # w1 dead after D/SA
    v.cody_waite_cascade(qA, rcA, gD4, 1.0, 0.0, 0.0)    # rc - gD4
    qAv = qA.rearrange("p (i k t) -> p i k t", i=8, k=8)

    # trees over k into unified 16-slot lo/hi (A: slots 0:8, B: 8:16)
    # scratch lives in buffers dead at tree time (NOT raw, so the next
    # chunk's input DMA can issue as soon as construction reads finish)
    t32a = D[:, 0:32 * T].rearrange("p (i k t) -> p i k t", i=8, k=4)
    mid = small("mid", n=16)
    m16 = mid.rearrange("p (i k t) -> p i k t", i=8, k=2)
    lo16 = small("lo16", n=16)
    hi16 = small("hi16", n=16)
    lo16v = lo16.rearrange("p (s t) -> p s t", s=16)
    hi16v = hi16.rearrange("p (s t) -> p s t", s=16)
    v.tensor_tensor(t32a, qAv[:, :, 0:4, :], qAv[:, :, 4:8, :], AOT.max)
    v.tensor_tensor(m16, t32a[:, :, 0:2, :], t32a[:, :, 2:4, :], AOT.max)
    v.tensor_tensor(lo16v[:, 0:8, :], m16[:, :, 0, :], m16[:, :, 1, :], AOT.max)
    v.tensor_tensor(t32a, qAv[:, :, 0:4, :], qAv[:, :, 4:8, :], AOT.min)
    v.tensor_tensor(m16, t32a[:, :, 0:2, :], t32a[:, :, 2:4, :], AOT.min)
    v.tensor_tensor(hi16v[:, 0:8, :], m16[:, :, 0, :], m16[:, :, 1, :], AOT.min)

    # ---------------- B-side windows (negated at write: q = -(rc'-2sg)) ----
    ratioB = sbp[:, 0:64 * T]                       # SB dead after ratioA & sbf
    E("rb", "v").tensor_tensor(
        ratioB.rearrange("p (i k t) -> p i k t", i=8, k=8), SA, dn(R), AOT.mult)
    rcBp = D                                        # rcA dead after qA
    E("rcb", "v").tensor_scalar(rcBp, ratioB, -1.0, 0.0, AOT.max, AOT.min)
    qB = Bb                                         # ratioA dead after rcA
    v.ln_bwd_dx(qB, rcBp, gD4, 1.0, -4.0, -1.0)     # qB = -rc' + gD4 - 4
    qBv = qB.rearrange("p (i k t) -> p i k t", i=8, k=8)

    # trees over i (same polarity as A side now)
    t32b = gD4[:, 0:32 * T].rearrange("p (i k t) -> p i k t", i=4, k=8)
    m16b = mid.rearrange("p (i k t) -> p i k t", i=2, k=8)
    v.tensor_tensor(t32b, qBv[:, 0:4, :, :], qBv[:, 4:8, :, :], AOT.max)
    v.tensor_tensor(m16b, t32b[:, 0:2, :, :], t32b[:, 2:4, :, :], AOT.max)
    v.tensor_tensor(lo16v[:, 8:16, :], m16b[:, 0, :, :], m16b[:, 1, :, :], AOT.max)
    v.tensor_tensor(t32b, qBv[:, 0:4, :, :], qBv[:, 4:8, :, :], AOT.min)
    v.tensor_tensor(m16b, t32b[:, 0:2, :, :], t32b[:, 2:4, :, :], AOT.min)
    v.tensor_tensor(hi16v[:, 8:16, :], m16b[:, 0, :, :], m16b[:, 1, :, :], AOT.min)

    # unified widths: w = max(min(hi+2,1) - max(lo-2,0), 0); iw = w*adj
    x1 = small("x1", n=16)
    x2 = small("x2", n=16)
    iw = small("iw16", n=16)
    x3 = iw                    # WAR-safe: iw's own write reads only x4/adjAB
    x4 = small("x4", n=16)
    E("w1p", "v").tensor_scalar(x1, lo16, 0.0, None, AOT.max)   # lo2
    v.tensor_scalar(x2, hi16, 4.0, 1.0, AOT.add, AOT.min)       # hi2
    v.tensor_tensor(x3, x2, x1, AOT.subtract)
    v.tensor_scalar(x4, x3, 0.0, None, AOT.max)
    iwv = iw.rearrange("p (s t) -> p s t", s=16)
    v.tensor_tensor(iw, x4, adjAB, AOT.mult)
    isum = tiny("isum")
    v.tensor_reduce(isum, iw.rearrange("p (s t) -> p t s", s=16),
                    axis=mybir.AxisListType.X, op=AOT.add)

    # ---------------- hull: bridges via sign algebra ----------------
    sAm = safv[:, 0:8, :, :]; sAf = safv[:, 1:9, :, :]
    sBm = sbfv[:, :, 0:8, :]; sBf = sbfv[:, :, 1:9, :]
    s1 = R                                          # R dead after ratioB
    E("s1", "v").tensor_tensor(dn(s1), sAm, sBf, AOT.add)
    s2 = sap[:, 0:64 * T]                           # SA dead after ratioB & saf
    E("s2", "v").tensor_tensor(s2.rearrange("p (i k t) -> p i k t", i=8, k=8),
                               sAf, sBm, AOT.add)
    u = gD4                                         # gD4 dead after qB
    E("u", "v").tensor_tensor(u, s1, s2, AOT.subtract)
    c2t = D                                         # rcBp dead after qB
    E("c2", "v").tensor_scalar(c2t, u, 1.0, -1.0, AOT.min, AOT.max)
    dd = Bb                                         # qB dead after trees
    E("dd", "v").tensor_tensor(dd, u, c2t, AOT.subtract)
    cM = s1                                         # s1 dead after u
    E("cm", "v").tensor_tensor(dn(cM), dd.rearrange("p (i k t) -> p i k t", i=8, k=8),
                               cp[:, 0:8, 0:8, :], AOT.mult)
    # redBR = sum_{i,k} cM  (tree + small reduce)
    cMv = cM.rearrange("p (i k t) -> p i k t", i=8, k=8)
    t32r = D[:, 0:32 * T].rearrange("p (i k t) -> p i k t", i=8, k=4)
    v.tensor_tensor(t32r, cMv[:, :, 0:4, :], cMv[:, :, 4:8, :], AOT.add)
    v.tensor_tensor(m16, t32r[:, :, 0:2, :], t32r[:, :, 2:4, :], AOT.add)
    r8 = lo16[:, 0:8 * T]                           # reuse small
    v.tensor_tensor(r8.rearrange("p (i t) -> p i t", i=8),
                    m16[:, :, 0, :], m16[:, :, 1, :], AOT.add)
    redBR = tiny("redbr")
    v.tensor_reduce(redBR, r8.rearrange("p (s t) -> p t s", s=8),
                    axis=mybir.AxisListType.X, op=AOT.add)

    # ---------------- hull: surviving edges ----------------
    # okEA = all_k sAf >= 0  ->  min-tree over k of sAf (f16 +-1)
    tr16a = gD4[:, 32 * T:48 * T].bitcast(F16).rearrange("p (i k t) -> p i k t", i=8, k=4)
    md16 = mid[:, 0:8 * T].bitcast(F16).rearrange("p (i k t) -> p i k t", i=8, k=2)
    mnsA = va1.bitcast(F16)[:, 0:8 * T]             # va1 dead after adjacency
    v.tensor_tensor(tr16a, sAf[:, :, 0:4, :], sAf[:, :, 4:8, :], AOT.min)
    v.tensor_tensor(md16, tr16a[:, :, 0:2, :], tr16a[:, :, 2:4, :], AOT.min)
    v.tensor_tensor(mnsA.rearrange("p (i t) -> p i t", i=8),
                    md16[:, :, 0, :], md16[:, :, 1, :], AOT.min)
    hw = iw                                         # reuse iw16 after isum
    hwv = hw.rearrange("p (s t) -> p s t", s=16)
    v.tensor_tensor(hwv[:, 0:8, :], mnsA.rearrange("p (i t) -> p i t", i=8),
                    adjA_c, AOT.mult)
    tr16b = gD4[:, 32 * T:48 * T].bitcast(F16).rearrange("p (i k t) -> p i k t", i=4, k=8)
    md16b = mid[:, 0:8 * T].bitcast(F16).rearrange("p (i k t) -> p i k t", i=2, k=8)
    v.tensor_tensor(tr16b, sBf[:, 0:4, :, :], sBf[:, 4:8, :, :], AOT.min)
    v.tensor_tensor(md16b, tr16b[:, 0:2, :, :], tr16b[:, 2:4, :, :], AOT.min)
    v.tensor_tensor(mnsA.rearrange("p (k t) -> p k t", k=8),
                    md16b[:, 0, :, :], md16b[:, 1, :, :], AOT.min)
    v.tensor_tensor(hwv[:, 8:16, :], mnsA.rearrange("p (k t) -> p k t", k=8),
                    adjB_c, AOT.mult)
    h1 = tiny("h1")
    v.tensor_reduce(h1, hw.rearrange("p (s t) -> p t s", s=16),
                    axis=mybir.AxisListType.X, op=AOT.add)

    # ---------------- per-item finals ----------------
    asum = tiny("asum")
    v.tensor_reduce(asum, adjAB.rearrange("p (s t) -> p t s", s=16),
                    axis=mybir.AxisListType.X, op=AOT.add)
    hsum = tiny("hsum")
    v.tensor_tensor(hsum, h1, redBR, AOT.add)       # = 2*hull_area
    u2 = tiny("u2")
    v.tensor_tensor(u2, asum, isum, AOT.subtract)   # = 2*union
    ru2 = tiny("ru2")
    v.reciprocal_approx_fast(out=ru2, in_=u2)
    rh = tiny("rh")
    v.reciprocal_approx_fast(out=rh, in_=hsum)
    iou = tiny("iou")
    v.tensor_tensor(iou, isum, ru2, AOT.mult)
    t3 = tiny("t3")
    v.tensor_tensor(t3, u2, rh, AOT.mult)
    ciou = tiny("ciou")
    v.affine_then_add(ciou, iou, t3, scale=1.0, bias=-1.0)
    out_view = out_d[ch * CH:(ch + 1) * CH].rearrange("(p t) -> p t", p=P)
    nc.sync.dma_start(out_view, ciou)


_CACHE = {}
_ASSIGN_DEFAULT = {"pads": "s"}


def _get_executable(npass=1):
    key = ("exec", npass)
    if key in _CACHE:
        return _CACHE[key]
    import jax
    from jax.sharding import Mesh, PartitionSpec, NamedSharding
    from jax.experimental.shard_map import shard_map
    from concourse import bass2jax

    nc = build_program(assign=_ASSIGN_DEFAULT, npass=npass)
    bass2jax.install_neuronx_cc_hook()

    partition_name = nc.partition_id_tensor.name if nc.partition_id_tensor else None
    in_names, in_shapes, out_names, out_avals = [], [], [], []
    for alloc in nc.m.functions[0].allocations:
        if not isinstance(alloc, mybir.MemoryLocationSet):
            continue
        name = alloc.memorylocations[0].name
        if alloc.kind == "ExternalInput":
            if name != partition_name:
                in_names.append(name)
                in_shapes.append((tuple(alloc.tensor_shape), mybir.dt.np(alloc.dtype)))
        elif alloc.kind == "ExternalOutput":
            out_names.append(name)
            out_avals.append(jax.core.ShapedArray(
                tuple(alloc.tensor_shape), mybir.dt.np(alloc.dtype)))
    all_names = in_names + out_names
    if partition_name is not None:
        all_names = all_names + [partition_name]

    def _body(*args):
        operands = list(args)
        if partition_name is not None:
            operands.append(bass2jax.partition_id_tensor())
        outs = bass2jax._bass_exec_p.bind(
            *operands,
            out_avals=tuple(out_avals),
            in_names=tuple(all_names),
            out_names=tuple(out_names),
            lowering_input_output_aliases=(),
            sim_require_finite=True,
            sim_require_nnan=True,
            nc=nc,
        )
        return tuple(outs)

    devices = jax.devices()[:NCORES]
    mesh = Mesh(np.asarray(devices), ("core",))
    nin = len(in_names)
    nout = len(out_names)
    sh = NamedSharding(mesh, PartitionSpec("core"))
    jf = shard_map(_body, mesh=mesh,
                   in_specs=(PartitionSpec("core"),) * (nin + nout),
                   out_specs=(PartitionSpec("core"),) * nout,
                   check_rep=False)

    def compile_fn():
        args = [jax.ShapeDtypeStruct((NCORES * s[0], *s[1:]), d, sharding=sh)
                for (s, d) in in_shapes]
        args += [jax.ShapeDtypeStruct((NCORES * av.shape[0], *av.shape[1:]),
                                      av.dtype, sharding=sh)
                 for av in out_avals]
        return jax.jit(jf, keep_unused=True).lower(*args).compile()

    try:
        sharded = bass2jax.fast_dispatch_compile(compile_fn)
    except Exception:
        sharded = jax.jit(jf, keep_unused=True)
    zeros = [np.zeros((NCORES * av.shape[0], *av.shape[1:]), av.dtype)
             for av in out_avals]
    _CACHE[key] = (sharded, sh, zeros)
    return _CACHE[key]


def kernel(a: np.ndarray, b: np.ndarray) -> np.ndarray:
    import jax
    a8 = np.asarray(a, dtype=np.float32).reshape(NCORES, NI, 16)
    b8 = np.asarray(b, dtype=np.float32).reshape(NCORES, NI, 16)
    ab = np.ascontiguousarray(np.concatenate([a8, b8], axis=2))
    sharded, sh, zeros = _get_executable()
    ab_dev = jax.device_put(ab.reshape(NCORES * NI, 32), sh)
    zeros_dev = [jax.device_put(z, sh) for z in zeros]
    import time as _time
    ciou = None
    for attempt, delay in enumerate((0, 5, 20)):
        if delay:
            _time.sleep(delay)
        try:
            out = sharded(ab_dev, *zeros_dev)
            ciou = np.asarray(out[0], dtype=np.float64)
            break
        except Exception:
            if attempt == 2:
                raise
    return np.float32(ciou.sum() / B)
